# revision 20
# baseline (speedup 1.0000x reference)
"""Trainium2 Bass kernel for BiDecoder edge dot products.

out[e] = dot(ufeat[src[e]], ifeat[dst[e]])   for E=300000 edges, D=256.

Strategy (8 NeuronCores, SPMD, v4 = dst-slab + PE one-hot expansion, flat gather):
  - Edges are globally sorted by dst; each core takes a contiguous 37500-edge
    slice, grouped into chunks of <=128 edges touching <=32 distinct cells.
    The host packs each chunk's distinct cells' ifeat rows (bf16) into a
    per-chunk 32-row window of a slab (3 chunks share a 128-partition block at
    base partitions 0/32/64 - the PE's allowed stationary bases), so the whole
    ifeat side needs NO per-edge gather, just a contiguous ~5MB slab load.
  - hu rows are gathered per-edge from HBM in bf16 (512B rows) with flat
    (transpose=False) SWDGE dma_gather on 4 queues. (Transpose-mode gathers
    corrupt data when calls run concurrently across queues - measured - so
    the kernel keeps hu in [edge, d] layout and never transposes it.)
  - PE: per chunk, HVexp[e, :] = onehot^T @ window  (K=32 window rows,
    M=128 edges, N=256 features) - one matmul expands the chunk's hv rows
    into PSUM, selecting each edge's own dst row.
  - DVE: per chunk, affine_mul_reduce(hu * HVexp) accumulates the per-edge
    dot into osb[:, c].
  - All cross-engine handoffs keep >=1 call/chunk of slack beyond the exact
    dependency boundary: semaphore updates can lead actual engine progress
    (measured on HW), so exact-boundary pipelining corrupts.
  - Host reorders y back to original edge order.
"""

import sys

for _p in ("/opt/trn_rl_repo",):
    if _p not in sys.path:
        sys.path.append(_p)

import numpy as np

P = 128
D = 256
E = 300000
NCORES = 8
ECORE = E // NCORES
N_GENE = 20000
N_CELL = 50000
W = 32                   # distinct cells per chunk window
CPC = 16                 # chunks per gather call (2048 rows/call)
NSLOT = 8                # gather slots (multiple of 4: slot sem <-> SWDGE queue binding)
NPS = 6                  # PSUM expansion tiles in rotation
NPROD = 4                # DVE product scratch tiles
NSTRIPE = 8              # slab/onehot load stripes

_PROGRAM_CACHE: dict = {}


def _cdiv(a, b):
    return -(-a // b)


def _wrap_idx(idx_i16: np.ndarray) -> np.ndarray:
    """[n] int16 -> [128, n/16] dma_gather idx layout (16-wrapped, 8x tiled)."""
    n = idx_i16.shape[0]
    assert n % 128 == 0
    cols = n // 16
    w = idx_i16.reshape(cols, 16).T
    return np.ascontiguousarray(np.tile(w, (8, 1)))


def _build_program(nchunk):
    import concourse.bacc as bacc
    import concourse.mybir as mybir
    from concourse.library_config import mlp

    assert nchunk % CPC == 0
    ncall = nchunk // CPC
    g3 = _cdiv(nchunk, 3)
    gs = _cdiv(g3, NSTRIPE)
    bf16 = mybir.dt.bfloat16
    f32 = mybir.dt.float32

    nc = bacc.Bacc("TRN2", debug=False, num_swdge_queues=4,
                   dynamic_dma_scratch_size=16384)
    ufeat = nc.dram_tensor("ufeat", [N_GENE, D], bf16, kind="ExternalInput")
    slab = nc.dram_tensor("slab", [96, g3, D], bf16, kind="ExternalInput")
    oht = nc.dram_tensor("oht", [96, g3, P], mybir.dt.float8e4,
                         kind="ExternalInput")
    sidx = nc.dram_tensor("sidx", [P, nchunk * P // 16], mybir.dt.int16,
                          kind="ExternalInput")
    y = nc.dram_tensor("y", [P, nchunk], f32, kind="ExternalOutput")

    with (
        nc.sbuf_tensor("hu", [P, NSLOT, CPC, D], bf16) as hu,
        nc.sbuf_tensor("slab_sb", [P, g3, D], bf16) as slab_sb,
        nc.sbuf_tensor("oht_sb", [P, g3, P], mybir.dt.float8e4) as oht_sb,
        nc.sbuf_tensor("sidx_sb", [P, nchunk * P // 16], mybir.dt.int16) as sidx_sb,
        nc.sbuf_tensor("prod", [P, NPROD, D], bf16) as prod,
        nc.sbuf_tensor("osb", [P, nchunk], f32) as osb,
        nc.psum_tensor("hv0", [P, D], f32) as hv0,
        nc.psum_tensor("hv1", [P, D], f32) as hv1,
        nc.psum_tensor("hv2", [P, D], f32) as hv2,
        nc.psum_tensor("hv3", [P, D], f32) as hv3,
        nc.psum_tensor("hv4", [P, D], f32) as hv4,
        nc.psum_tensor("hv5", [P, D], f32) as hv5,
        nc.psum_tensor("hvdum", [P, D], f32) as hvdum,
        nc.semaphore("io_s") as io_s,
        nc.semaphore("io") as io,
        nc.semaphore("pec") as pec,
        nc.semaphore("dvec") as dvec,
        nc.semaphore("io2") as io2,
        nc.Block() as block,
        __import__("contextlib").ExitStack() as _stk,
    ):
        gu = [_stk.enter_context(nc.semaphore(f"gu{i}")) for i in range(NSLOT)]
        hvps = [hv0, hv1, hv2, hv3, hv4, hv5]

        @block.gpsimd
        def _(gp):
            gp.load_library(mlp)
            gp.wait_ge(io_s, 16)  # sidx loaded
            for k in range(ncall):
                s = k % NSLOT
                if k >= NSLOT - 1:
                    # slot freed when DVE consumed call k-NSLOT; +1 call slack
                    gp.wait_ge(dvec, CPC * (k - NSLOT + 2))
                cols = slice(k * CPC * P // 16, (k + 1) * CPC * P // 16)
                gp.dma_gather(
                    hu[:, s], ufeat[:, :], sidx_sb[:, cols],
                    CPC * P, CPC * P, D,
                    transpose=False, queue_num=k % 4, single_packet=False,
                ).then_inc(gu[s], 16)
            for s in range(NSLOT):
                cnt = (ncall - s + NSLOT - 1) // NSLOT
                if cnt:
                    gp.wait_ge(gu[s], 16 * cnt)

        @block.tensor
        def _(pe):
            laststripe = -1
            for c in range(nchunk):
                stripe = (c // 3) // gs
                if stripe > laststripe:
                    pe.wait_ge(io, 32 * (stripe + 1))
                    laststripe = stripe
                if c >= NPS:
                    # psum tile c%NPS freed when DVE did chunk c-NPS; +1 slack
                    pe.wait_ge(dvec, max(1, c - NPS + 2))
                p0 = 32 * (c % 3)
                g = c // 3
                pe.matmul(
                    out=hvps[c % NPS][:, :],
                    lhsT=oht_sb[p0 : p0 + W, g, :],
                    rhs=slab_sb[p0 : p0 + W, g, :],
                    start=True, stop=True,
                ).then_inc(pec, 1)
            for _ in range(4):
                pe.matmul(
                    out=hvdum[:, :],
                    lhsT=oht_sb[0:W, 0, :],
                    rhs=slab_sb[0:W, 0, :],
                    start=True, stop=True,
                ).then_inc(pec, 1)

        @block.vector
        def _(v):
            for c in range(nchunk):
                k, r = divmod(c, CPC)
                s = k % NSLOT
                if r == 0:
                    v.wait_ge(gu[s], 16 * (k // NSLOT + 1))
                # chunk c's expansion executed: PE issued through c+2 (slack)
                v.wait_ge(pec, c + 3)
                if c >= NPROD:
                    v.wait_ge(dvec, c - NPROD + 1)  # prod scratch WAR self-gate
                v.affine_mul_reduce(
                    out=prod[:, c % NPROD, :],
                    accum_out=osb[:, c : c + 1],
                    in0=hu[:, s, r, :],
                    in1=hvps[c % NPS][:, :],
                    scale=1.0,
                    bias=0.0,
                ).then_inc(dvec, 1)

        @block.scalar
        def _(sc):
            sc.dma_start(sidx_sb[:], sidx[:]).then_inc(io_s, 16)

        @block.sync
        def _(sy):
            # striped slab/onehot loads: PE starts after stripe 0; stripes
            # are issued serially so the shared io thresholds stay ordered
            for i in range(NSTRIPE):
                if i > 0:
                    sy.wait_ge(io, 32 * i)
                cs = slice(i * gs, min(g3, (i + 1) * gs))
                sy.dma_start(slab_sb[0:96, cs], slab[:, cs]).then_inc(io, 16)
                sy.dma_start(oht_sb[0:96, cs], oht[:, cs]).then_inc(io, 16)
            sy.wait_ge(dvec, nchunk)
            sy.dma_start(y[:, :], osb[:, :]).then_inc(io2, 16)
            sy.wait_ge(io2, 16)

    nc.compile()
    return nc


def _chunk_core(s_j, d_j, ids_j):
    """Greedy chunks: <=128 edges, <=W distinct (sorted) dst cells each.

    Returns (srcs [n,128] int16 pad0, cells [n,W] int32 pad0,
             j [n,128] int8 pad-1, eids [n,128] int64 pad-1).
    """
    n = len(d_j)
    newcell = np.empty(n, bool)
    newcell[0] = True
    np.not_equal(d_j[1:], d_j[:-1], out=newcell[1:])
    cellrank = np.cumsum(newcell) - 1
    chunks = []
    i = 0
    while i < n:
        base_rank = cellrank[i]
        hi = min(n, i + 128)
        if cellrank[hi - 1] - base_rank >= W:
            hi = int(np.searchsorted(cellrank[i:hi], base_rank + W)) + i
        chunks.append((i, hi))
        i = hi
    m = len(chunks)
    srcs = np.zeros((m, P), np.int16)
    cells = np.zeros((m, W), np.int32)
    jloc = np.full((m, P), -1, np.int8)
    eids = np.full((m, P), -1, np.int64)
    for ci, (lo, hi) in enumerate(chunks):
        k = hi - lo
        srcs[ci, :k] = s_j[lo:hi]
        jloc[ci, :k] = (cellrank[lo:hi] - cellrank[lo]).astype(np.int8)
        eids[ci, :k] = ids_j[lo:hi]
        cc = np.unique(d_j[lo:hi])
        cells[ci, : len(cc)] = cc
    return srcs, cells, jloc, eids


def kernel(ufeat, ifeat, src, dst):
    import ml_dtypes
    from concourse.bass_utils import run_bass_kernel_spmd

    bf16 = ml_dtypes.bfloat16
    ufeat_bf = np.ascontiguousarray(np.asarray(ufeat, dtype=np.float32)).astype(bf16)
    ifeat_bf = np.ascontiguousarray(np.asarray(ifeat, dtype=np.float32)).astype(bf16)
    src_f = np.asarray(src).ravel().astype(np.int64)
    dst_f = np.asarray(dst).ravel().astype(np.int64)
    assert src_f.shape == (E,) and dst_f.shape == (E,)

    order = np.argsort(dst_f, kind="stable")
    per_core = []
    for jc in range(NCORES):
        sl = order[jc * ECORE : (jc + 1) * ECORE]
        per_core.append(_chunk_core(src_f[sl], dst_f[sl], sl))

    nchunk = _cdiv(max(pc[0].shape[0] for pc in per_core), CPC) * CPC
    g3 = _cdiv(nchunk, 3)

    if nchunk not in _PROGRAM_CACHE:
        _PROGRAM_CACHE[nchunk] = _build_program(nchunk)
    nc = _PROGRAM_CACHE[nchunk]

    in_maps = []
    eid_list = []
    for jc in range(NCORES):
        srcs, cells, jloc, eids = per_core[jc]
        m = srcs.shape[0]
        pad3 = g3 * 3  # chunk slots incl. packing remainder
        if m < pad3:
            pad = pad3 - m
            srcs = np.vstack([srcs, np.zeros((pad, P), np.int16)])
            cells = np.vstack([cells, np.zeros((pad, W), np.int32)])
            jloc = np.vstack([jloc, np.full((pad, P), -1, np.int8)])
            eids = np.vstack([eids, np.full((pad, P), -1, np.int64)])
        # slab: [96, g3, 256]; chunk c -> partitions 32*(c%3)+w, column c//3
        rows = ifeat_bf[cells.ravel()].reshape(pad3, W, D)
        slabT = np.ascontiguousarray(
            rows.reshape(g3, 3, W, D).transpose(1, 2, 0, 3).reshape(96, g3, D)
        )
        # onehot^T: [96, g3, 128]; oht[32*(c%3)+w, c//3, e] = (jloc[c,e] == w)
        ohb = (
            jloc[:, :, None] == np.arange(W, dtype=np.int8)[None, None, :]
        ).transpose(0, 2, 1)                       # [pad3, W, 128]
        ohT = np.ascontiguousarray(
            ohb.reshape(g3, 3, W, P).transpose(1, 2, 0, 3).reshape(96, g3, P)
        ).astype(ml_dtypes.float8_e4m3)
        sidx_w = _wrap_idx(srcs[:nchunk].ravel())
        in_maps.append(
            {"ufeat": ufeat_bf, "slab": slabT, "oht": ohT, "sidx": sidx_w}
        )
        eid_list.append(eids[:nchunk])

    res = run_bass_kernel_spmd(nc, in_maps, core_ids=list(range(NCORES)))

    out = np.empty((E, 1), np.float32)
    for jc in range(NCORES):
        yj = np.asarray(res.results[jc]["y"])  # [128, nchunk]
        eids = eid_list[jc]                    # [nchunk, 128]
        m = eids >= 0
        out[eids[m], 0] = yj.T[m]
    return out


# revision 21
# speedup vs baseline: 1.1698x; 1.1698x over previous
"""Trainium2 Bass kernel for BiDecoder edge dot products.

out[e] = dot(ufeat[src[e]], ifeat[dst[e]])   for E=300000 edges, D=256.

Strategy (8 NeuronCores, SPMD, v4 = dst-slab + PE one-hot expansion, flat gather):
  - Edges are globally sorted by dst; each core takes a contiguous 37500-edge
    slice, grouped into chunks of <=128 edges touching <=32 distinct cells.
    The host packs each chunk's distinct cells' ifeat rows (bf16) into a
    per-chunk 32-row window of a slab (3 chunks share a 128-partition block at
    base partitions 0/32/64 - the PE's allowed stationary bases), so the whole
    ifeat side needs NO per-edge gather, just a contiguous ~5MB slab load.
  - hu rows are gathered per-edge from HBM in bf16 (512B rows) with flat
    (transpose=False) SWDGE dma_gather on 4 queues. (Transpose-mode gathers
    corrupt data when calls run concurrently across queues - measured - so
    the kernel keeps hu in [edge, d] layout and never transposes it.)
  - PE: per chunk, HVexp[e, :] = onehot^T @ window  (K=32 window rows,
    M=128 edges, N=256 features) - one matmul expands the chunk's hv rows
    into PSUM, selecting each edge's own dst row.
  - DVE: per chunk, affine_mul_reduce(hu * HVexp) accumulates the per-edge
    dot into osb[:, c].
  - All cross-engine handoffs keep >=1 call/chunk of slack beyond the exact
    dependency boundary: semaphore updates can lead actual engine progress
    (measured on HW), so exact-boundary pipelining corrupts.
  - Host reorders y back to original edge order.
"""

import sys

for _p in ("/opt/trn_rl_repo",):
    if _p not in sys.path:
        sys.path.append(_p)

import numpy as np

P = 128
D = 256
E = 300000
NCORES = 8
ECORE = E // NCORES
N_GENE = 20000
N_CELL = 50000
W = 32                   # distinct cells per chunk window
CPC = 8                  # chunks per gather call (1024 rows/call)
NSLOT = 8                # gather slots (multiple of 4: slot sem <-> SWDGE queue binding)
NPS = 6                  # PSUM expansion tiles in rotation
NPROD = 4                # DVE product scratch tiles
NSTRIPE = 8              # slab/onehot load stripes

_PROGRAM_CACHE: dict = {}


def _cdiv(a, b):
    return -(-a // b)


def _wrap_idx(idx_i16: np.ndarray) -> np.ndarray:
    """[n] int16 -> [128, n/16] dma_gather idx layout (16-wrapped, 8x tiled)."""
    n = idx_i16.shape[0]
    assert n % 128 == 0
    cols = n // 16
    w = idx_i16.reshape(cols, 16).T
    return np.ascontiguousarray(np.tile(w, (8, 1)))


def _build_program(nchunk):
    import concourse.bacc as bacc
    import concourse.mybir as mybir
    from concourse.library_config import mlp

    assert nchunk % CPC == 0
    ncall = nchunk // CPC
    g3 = _cdiv(nchunk, 3)
    gs = _cdiv(g3, NSTRIPE)
    bf16 = mybir.dt.bfloat16
    f32 = mybir.dt.float32

    nc = bacc.Bacc("TRN2", debug=False, num_swdge_queues=4,
                   dynamic_dma_scratch_size=32768)
    ufeat = nc.dram_tensor("ufeat", [N_GENE, D], bf16, kind="ExternalInput")
    slab = nc.dram_tensor("slab", [96, g3, D], bf16, kind="ExternalInput")
    oht = nc.dram_tensor("oht", [96, g3, P], mybir.dt.float8e4,
                         kind="ExternalInput")
    sidx = nc.dram_tensor("sidx", [P, nchunk * P // 16], mybir.dt.int16,
                          kind="ExternalInput")
    y = nc.dram_tensor("y", [P, nchunk], f32, kind="ExternalOutput")

    with (
        nc.sbuf_tensor("hu", [P, NSLOT, CPC, D], bf16) as hu,
        nc.sbuf_tensor("slab_sb", [P, g3, D], bf16) as slab_sb,
        nc.sbuf_tensor("oht_sb", [P, g3, P], mybir.dt.float8e4) as oht_sb,
        nc.sbuf_tensor("sidx_sb", [P, nchunk * P // 16], mybir.dt.int16) as sidx_sb,
        nc.sbuf_tensor("prod", [P, NPROD, D], bf16) as prod,
        nc.sbuf_tensor("osb", [P, nchunk], f32) as osb,
        nc.psum_tensor("hv0", [P, D], f32) as hv0,
        nc.psum_tensor("hv1", [P, D], f32) as hv1,
        nc.psum_tensor("hv2", [P, D], f32) as hv2,
        nc.psum_tensor("hv3", [P, D], f32) as hv3,
        nc.psum_tensor("hv4", [P, D], f32) as hv4,
        nc.psum_tensor("hv5", [P, D], f32) as hv5,
        nc.psum_tensor("hvdum", [P, D], f32) as hvdum,
        nc.semaphore("io_s") as io_s,
        nc.semaphore("io") as io,
        nc.semaphore("pec") as pec,
        nc.semaphore("dvec") as dvec,
        nc.semaphore("io2") as io2,
        nc.Block() as block,
        __import__("contextlib").ExitStack() as _stk,
    ):
        gu = [_stk.enter_context(nc.semaphore(f"gu{i}")) for i in range(NSLOT)]
        hvps = [hv0, hv1, hv2, hv3, hv4, hv5]

        @block.gpsimd
        def _(gp):
            gp.load_library(mlp)
            idxcols = nchunk * P // 16
            q4 = _cdiv(idxcols, 4)
            lastq = -1
            for k in range(ncall):
                kq = min(3, ((k + 1) * CPC * P // 16 - 1) // q4)
                if kq > lastq:
                    gp.wait_ge(io_s, 16 * (kq + 1))
                    lastq = kq
                s = k % NSLOT
                if k >= NSLOT - 1:
                    # slot freed when DVE consumed call k-NSLOT; +1 call slack
                    gp.wait_ge(dvec, CPC * (k - NSLOT + 2))
                cols = slice(k * CPC * P // 16, (k + 1) * CPC * P // 16)
                gp.dma_gather(
                    hu[:, s], ufeat[:, :], sidx_sb[:, cols],
                    CPC * P, CPC * P, D,
                    transpose=False, queue_num=k % 4, single_packet=False,
                ).then_inc(gu[s], 16)
            for s in range(NSLOT):
                cnt = (ncall - s + NSLOT - 1) // NSLOT
                if cnt:
                    gp.wait_ge(gu[s], 16 * cnt)

        @block.tensor
        def _(pe):
            laststripe = -1
            for c in range(nchunk):
                stripe = (c // 3) // gs
                if stripe > laststripe:
                    pe.wait_ge(io, 32 * (stripe + 1))
                    laststripe = stripe
                if c >= NPS:
                    # psum tile c%NPS freed when DVE did chunk c-NPS; +1 slack
                    pe.wait_ge(dvec, max(1, c - NPS + 2))
                p0 = 32 * (c % 3)
                g = c // 3
                pe.matmul(
                    out=hvps[c % NPS][:, :],
                    lhsT=oht_sb[p0 : p0 + W, g, :],
                    rhs=slab_sb[p0 : p0 + W, g, :],
                    start=True, stop=True,
                ).then_inc(pec, 1)
            for _ in range(4):
                pe.matmul(
                    out=hvdum[:, :],
                    lhsT=oht_sb[0:W, 0, :],
                    rhs=slab_sb[0:W, 0, :],
                    start=True, stop=True,
                ).then_inc(pec, 1)

        @block.vector
        def _(v):
            for c in range(nchunk):
                k, r = divmod(c, CPC)
                s = k % NSLOT
                if r == 0:
                    v.wait_ge(gu[s], 16 * (k // NSLOT + 1))
                # chunk c's expansion executed: PE issued through c+2 (slack)
                v.wait_ge(pec, c + 3)
                if c >= NPROD:
                    v.wait_ge(dvec, c - NPROD + 1)  # prod scratch WAR self-gate
                v.affine_mul_reduce(
                    out=prod[:, c % NPROD, :],
                    accum_out=osb[:, c : c + 1],
                    in0=hu[:, s, r, :],
                    in1=hvps[c % NPS][:, :],
                    scale=1.0,
                    bias=0.0,
                ).then_inc(dvec, 1)

        @block.scalar
        def _(sc):
            idxcols = nchunk * P // 16
            q4 = _cdiv(idxcols, 4)
            for i in range(4):
                if i > 0:
                    sc.wait_ge(io_s, 16 * i)
                cs = slice(i * q4, min(idxcols, (i + 1) * q4))
                sc.dma_start(sidx_sb[:, cs], sidx[:, cs]).then_inc(io_s, 16)

        @block.sync
        def _(sy):
            # striped slab/onehot loads: PE starts after stripe 0; stripes
            # are issued serially so the shared io thresholds stay ordered
            for i in range(NSTRIPE):
                if i > 0:
                    sy.wait_ge(io, 32 * i)
                cs = slice(i * gs, min(g3, (i + 1) * gs))
                sy.dma_start(slab_sb[0:96, cs], slab[:, cs]).then_inc(io, 16)
                sy.dma_start(oht_sb[0:96, cs], oht[:, cs]).then_inc(io, 16)
            sy.wait_ge(dvec, nchunk)
            sy.dma_start(y[:, :], osb[:, :]).then_inc(io2, 16)
            sy.wait_ge(io2, 16)

    nc.compile()
    return nc


def _chunk_core(s_j, d_j, ids_j):
    """Greedy chunks: <=128 edges, <=W distinct (sorted) dst cells each.

    Returns (srcs [n,128] int16 pad0, cells [n,W] int32 pad0,
             j [n,128] int8 pad-1, eids [n,128] int64 pad-1).
    """
    n = len(d_j)
    newcell = np.empty(n, bool)
    newcell[0] = True
    np.not_equal(d_j[1:], d_j[:-1], out=newcell[1:])
    cellrank = np.cumsum(newcell) - 1
    chunks = []
    i = 0
    while i < n:
        base_rank = cellrank[i]
        hi = min(n, i + 128)
        if cellrank[hi - 1] - base_rank >= W:
            hi = int(np.searchsorted(cellrank[i:hi], base_rank + W)) + i
        chunks.append((i, hi))
        i = hi
    m = len(chunks)
    srcs = np.zeros((m, P), np.int16)
    cells = np.zeros((m, W), np.int32)
    jloc = np.full((m, P), -1, np.int8)
    eids = np.full((m, P), -1, np.int64)
    for ci, (lo, hi) in enumerate(chunks):
        k = hi - lo
        srcs[ci, :k] = s_j[lo:hi]
        jloc[ci, :k] = (cellrank[lo:hi] - cellrank[lo]).astype(np.int8)
        eids[ci, :k] = ids_j[lo:hi]
        cc = np.unique(d_j[lo:hi])
        cells[ci, : len(cc)] = cc
    return srcs, cells, jloc, eids


def kernel(ufeat, ifeat, src, dst):
    import ml_dtypes
    from concourse.bass_utils import run_bass_kernel_spmd

    bf16 = ml_dtypes.bfloat16
    ufeat_bf = np.ascontiguousarray(np.asarray(ufeat, dtype=np.float32)).astype(bf16)
    ifeat_bf = np.ascontiguousarray(np.asarray(ifeat, dtype=np.float32)).astype(bf16)
    src_f = np.asarray(src).ravel().astype(np.int64)
    dst_f = np.asarray(dst).ravel().astype(np.int64)
    assert src_f.shape == (E,) and dst_f.shape == (E,)

    order = np.argsort(dst_f, kind="stable")
    per_core = []
    for jc in range(NCORES):
        sl = order[jc * ECORE : (jc + 1) * ECORE]
        per_core.append(_chunk_core(src_f[sl], dst_f[sl], sl))

    nchunk = _cdiv(max(pc[0].shape[0] for pc in per_core), CPC) * CPC
    g3 = _cdiv(nchunk, 3)

    if nchunk not in _PROGRAM_CACHE:
        _PROGRAM_CACHE[nchunk] = _build_program(nchunk)
    nc = _PROGRAM_CACHE[nchunk]

    in_maps = []
    eid_list = []
    for jc in range(NCORES):
        srcs, cells, jloc, eids = per_core[jc]
        m = srcs.shape[0]
        pad3 = g3 * 3  # chunk slots incl. packing remainder
        if m < pad3:
            pad = pad3 - m
            srcs = np.vstack([srcs, np.zeros((pad, P), np.int16)])
            cells = np.vstack([cells, np.zeros((pad, W), np.int32)])
            jloc = np.vstack([jloc, np.full((pad, P), -1, np.int8)])
            eids = np.vstack([eids, np.full((pad, P), -1, np.int64)])
        # slab: [96, g3, 256]; chunk c -> partitions 32*(c%3)+w, column c//3
        rows = ifeat_bf[cells.ravel()].reshape(pad3, W, D)
        slabT = np.ascontiguousarray(
            rows.reshape(g3, 3, W, D).transpose(1, 2, 0, 3).reshape(96, g3, D)
        )
        # onehot^T: [96, g3, 128]; oht[32*(c%3)+w, c//3, e] = (jloc[c,e] == w)
        ohb = (
            jloc[:, :, None] == np.arange(W, dtype=np.int8)[None, None, :]
        ).transpose(0, 2, 1)                       # [pad3, W, 128]
        ohT = np.ascontiguousarray(
            ohb.reshape(g3, 3, W, P).transpose(1, 2, 0, 3).reshape(96, g3, P)
        ).astype(ml_dtypes.float8_e4m3)
        sidx_w = _wrap_idx(srcs[:nchunk].ravel())
        in_maps.append(
            {"ufeat": ufeat_bf, "slab": slabT, "oht": ohT, "sidx": sidx_w}
        )
        eid_list.append(eids[:nchunk])

    res = run_bass_kernel_spmd(nc, in_maps, core_ids=list(range(NCORES)))

    out = np.empty((E, 1), np.float32)
    for jc in range(NCORES):
        yj = np.asarray(res.results[jc]["y"])  # [128, nchunk]
        eids = eid_list[jc]                    # [nchunk, 128]
        m = eids >= 0
        out[eids[m], 0] = yj.T[m]
    return out


# revision 22
# speedup vs baseline: 1.1784x; 1.0073x over previous
"""Trainium2 Bass kernel for BiDecoder edge dot products.

out[e] = dot(ufeat[src[e]], ifeat[dst[e]])   for E=300000 edges, D=256.

Strategy (8 NeuronCores, SPMD, v4 = dst-slab + PE one-hot expansion, flat gather):
  - Edges are globally sorted by dst; each core takes a contiguous 37500-edge
    slice, grouped into chunks of <=128 edges touching <=32 distinct cells.
    The host packs each chunk's distinct cells' ifeat rows (bf16) into a
    per-chunk 32-row window of a slab (3 chunks share a 128-partition block at
    base partitions 0/32/64 - the PE's allowed stationary bases), so the whole
    ifeat side needs NO per-edge gather, just a contiguous ~5MB slab load.
  - hu rows are gathered per-edge from HBM in bf16 (512B rows) with flat
    (transpose=False) SWDGE dma_gather on 4 queues. (Transpose-mode gathers
    corrupt data when calls run concurrently across queues - measured - so
    the kernel keeps hu in [edge, d] layout and never transposes it.)
  - PE: per chunk, HVexp[e, :] = onehot^T @ window  (K=32 window rows,
    M=128 edges, N=256 features) - one matmul expands the chunk's hv rows
    into PSUM, selecting each edge's own dst row.
  - DVE: per chunk, affine_mul_reduce(hu * HVexp) accumulates the per-edge
    dot into osb[:, c].
  - All cross-engine handoffs keep >=1 call/chunk of slack beyond the exact
    dependency boundary: semaphore updates can lead actual engine progress
    (measured on HW), so exact-boundary pipelining corrupts.
  - Host reorders y back to original edge order.
"""

import sys

for _p in ("/opt/trn_rl_repo",):
    if _p not in sys.path:
        sys.path.append(_p)

import numpy as np

P = 128
D = 256
E = 300000
NCORES = 8
ECORE = E // NCORES
N_GENE = 20000
N_CELL = 50000
W = 32                   # distinct cells per chunk window
CPC = 8                  # chunks per gather call (1024 rows/call)
NSLOT = 8                # gather slots (multiple of 4: slot sem <-> SWDGE queue binding)
NPS = 6                  # PSUM expansion tiles in rotation
NPROD = 4                # DVE product scratch tiles
NSTRIPE = 8              # slab/onehot load stripes

_PROGRAM_CACHE: dict = {}


def _cdiv(a, b):
    return -(-a // b)


def _wrap_idx(idx_i16: np.ndarray) -> np.ndarray:
    """[n] int16 -> [128, n/16] dma_gather idx layout (16-wrapped, 8x tiled)."""
    n = idx_i16.shape[0]
    assert n % 128 == 0
    cols = n // 16
    w = idx_i16.reshape(cols, 16).T
    return np.ascontiguousarray(np.tile(w, (8, 1)))


def _build_program(nchunk):
    import concourse.bacc as bacc
    import concourse.mybir as mybir
    from concourse.library_config import mlp

    assert nchunk % CPC == 0
    ncall = nchunk // CPC
    g3 = _cdiv(nchunk, 3)
    gs = _cdiv(g3 - 2, NSTRIPE - 1)
    stripes = [(0, 2)] + [
        (2 + i * gs, min(g3, 2 + (i + 1) * gs)) for i in range(NSTRIPE - 1)
    ]
    stripe_of_col = {}
    for i, (c0, c1) in enumerate(stripes):
        for c in range(c0, c1):
            stripe_of_col[c] = i
    bf16 = mybir.dt.bfloat16
    f32 = mybir.dt.float32

    nc = bacc.Bacc("TRN2", debug=False, num_swdge_queues=4,
                   dynamic_dma_scratch_size=32768)
    ufeat = nc.dram_tensor("ufeat", [N_GENE, D], bf16, kind="ExternalInput")
    slab = nc.dram_tensor("slab", [96, g3, D], bf16, kind="ExternalInput")
    oht = nc.dram_tensor("oht", [96, g3, P], mybir.dt.float8e4,
                         kind="ExternalInput")
    sidx = nc.dram_tensor("sidx", [P, nchunk * P // 16], mybir.dt.int16,
                          kind="ExternalInput")
    y = nc.dram_tensor("y", [P, nchunk], f32, kind="ExternalOutput")

    with (
        nc.sbuf_tensor("hu", [P, NSLOT, CPC, D], bf16) as hu,
        nc.sbuf_tensor("slab_sb", [P, g3, D], bf16) as slab_sb,
        nc.sbuf_tensor("oht_sb", [P, g3, P], mybir.dt.float8e4) as oht_sb,
        nc.sbuf_tensor("sidx_sb", [P, nchunk * P // 16], mybir.dt.int16) as sidx_sb,
        nc.sbuf_tensor("prod", [P, NPROD, D], bf16) as prod,
        nc.sbuf_tensor("osb", [P, nchunk], f32) as osb,
        nc.psum_tensor("hv0", [P, D], f32) as hv0,
        nc.psum_tensor("hv1", [P, D], f32) as hv1,
        nc.psum_tensor("hv2", [P, D], f32) as hv2,
        nc.psum_tensor("hv3", [P, D], f32) as hv3,
        nc.psum_tensor("hv4", [P, D], f32) as hv4,
        nc.psum_tensor("hv5", [P, D], f32) as hv5,
        nc.psum_tensor("hvdum", [P, D], f32) as hvdum,
        nc.semaphore("io_s") as io_s,
        nc.semaphore("io") as io,
        nc.semaphore("pec") as pec,
        nc.semaphore("dvec") as dvec,
        nc.semaphore("io2") as io2,
        nc.Block(no_gpsimd_drain=True) as block,
        __import__("contextlib").ExitStack() as _stk,
    ):
        gu = [_stk.enter_context(nc.semaphore(f"gu{i}")) for i in range(NSLOT)]
        hvps = [hv0, hv1, hv2, hv3, hv4, hv5]

        @block.gpsimd
        def _(gp):
            gp.load_library(mlp)
            idxcols = nchunk * P // 16
            q4 = _cdiv(idxcols, 4)
            lastq = -1
            for k in range(ncall):
                kq = min(3, ((k + 1) * CPC * P // 16 - 1) // q4)
                if kq > lastq:
                    gp.wait_ge(io_s, 16 * (kq + 1))
                    lastq = kq
                s = k % NSLOT
                if k >= NSLOT - 1:
                    # slot freed when DVE consumed call k-NSLOT; +1 call slack
                    gp.wait_ge(dvec, CPC * (k - NSLOT + 2))
                cols = slice(k * CPC * P // 16, (k + 1) * CPC * P // 16)
                gp.dma_gather(
                    hu[:, s], ufeat[:, :], sidx_sb[:, cols],
                    CPC * P, CPC * P, D,
                    transpose=False, queue_num=k % 4, single_packet=False,
                ).then_inc(gu[s], 16)
            for s in range(NSLOT):
                cnt = (ncall - s + NSLOT - 1) // NSLOT
                if cnt:
                    gp.wait_ge(gu[s], 16 * cnt)

        @block.tensor
        def _(pe):
            laststripe = -1
            for c in range(nchunk):
                stripe = stripe_of_col[c // 3]
                if stripe > laststripe:
                    pe.wait_ge(io, 32 * (stripe + 1))
                    laststripe = stripe
                if c >= NPS:
                    # psum tile c%NPS freed when DVE did chunk c-NPS; +1 slack
                    pe.wait_ge(dvec, max(1, c - NPS + 2))
                p0 = 32 * (c % 3)
                g = c // 3
                pe.matmul(
                    out=hvps[c % NPS][:, :],
                    lhsT=oht_sb[p0 : p0 + W, g, :],
                    rhs=slab_sb[p0 : p0 + W, g, :],
                    start=True, stop=True,
                ).then_inc(pec, 1)
            for _ in range(4):
                pe.matmul(
                    out=hvdum[:, :],
                    lhsT=oht_sb[0:W, 0, :],
                    rhs=slab_sb[0:W, 0, :],
                    start=True, stop=True,
                ).then_inc(pec, 1)

        @block.vector
        def _(v):
            for c in range(nchunk):
                k, r = divmod(c, CPC)
                s = k % NSLOT
                if r == 0:
                    v.wait_ge(gu[s], 16 * (k // NSLOT + 1))
                # chunk c's expansion executed: PE issued through c+2 (slack)
                v.wait_ge(pec, c + 3)
                if c >= NPROD:
                    v.wait_ge(dvec, c - NPROD + 1)  # prod scratch WAR self-gate
                v.affine_mul_reduce(
                    out=prod[:, c % NPROD, :],
                    accum_out=osb[:, c : c + 1],
                    in0=hu[:, s, r, :],
                    in1=hvps[c % NPS][:, :],
                    scale=1.0,
                    bias=0.0,
                ).then_inc(dvec, 1)

        @block.scalar
        def _(sc):
            idxcols = nchunk * P // 16
            q4 = _cdiv(idxcols, 4)
            for i in range(4):
                if i > 0:
                    sc.wait_ge(io_s, 16 * i)
                cs = slice(i * q4, min(idxcols, (i + 1) * q4))
                sc.dma_start(sidx_sb[:, cs], sidx[:, cs]).then_inc(io_s, 16)

        @block.sync
        def _(sy):
            # striped slab/onehot loads: tiny first stripe so PE starts fast;
            # later stripes wait for DVE progress so their traffic never
            # competes with the early gather calls
            for i, (c0, c1) in enumerate(stripes):
                if i > 0:
                    sy.wait_ge(io, 32 * i)
                    sy.wait_ge(dvec, max(0, 8 * (i - 1)))
                cs = slice(c0, c1)
                sy.dma_start(slab_sb[0:96, cs], slab[:, cs]).then_inc(io, 16)
                sy.dma_start(oht_sb[0:96, cs], oht[:, cs]).then_inc(io, 16)
            sy.wait_ge(dvec, nchunk)
            sy.dma_start(y[:, :], osb[:, :]).then_inc(io2, 16)
            sy.wait_ge(io2, 16)

    nc.compile()
    return nc


def _chunk_core(s_j, d_j, ids_j):
    """Greedy chunks: <=128 edges, <=W distinct (sorted) dst cells each.

    Returns (srcs [n,128] int16 pad0, cells [n,W] int32 pad0,
             j [n,128] int8 pad-1, eids [n,128] int64 pad-1).
    """
    n = len(d_j)
    newcell = np.empty(n, bool)
    newcell[0] = True
    np.not_equal(d_j[1:], d_j[:-1], out=newcell[1:])
    cellrank = np.cumsum(newcell) - 1
    chunks = []
    i = 0
    while i < n:
        base_rank = cellrank[i]
        hi = min(n, i + 128)
        if cellrank[hi - 1] - base_rank >= W:
            hi = int(np.searchsorted(cellrank[i:hi], base_rank + W)) + i
        chunks.append((i, hi))
        i = hi
    m = len(chunks)
    srcs = np.zeros((m, P), np.int16)
    cells = np.zeros((m, W), np.int32)
    jloc = np.full((m, P), -1, np.int8)
    eids = np.full((m, P), -1, np.int64)
    for ci, (lo, hi) in enumerate(chunks):
        k = hi - lo
        srcs[ci, :k] = s_j[lo:hi]
        jloc[ci, :k] = (cellrank[lo:hi] - cellrank[lo]).astype(np.int8)
        eids[ci, :k] = ids_j[lo:hi]
        cc = np.unique(d_j[lo:hi])
        cells[ci, : len(cc)] = cc
    return srcs, cells, jloc, eids


def kernel(ufeat, ifeat, src, dst):
    import ml_dtypes
    from concourse.bass_utils import run_bass_kernel_spmd

    bf16 = ml_dtypes.bfloat16
    ufeat_bf = np.ascontiguousarray(np.asarray(ufeat, dtype=np.float32)).astype(bf16)
    ifeat_bf = np.ascontiguousarray(np.asarray(ifeat, dtype=np.float32)).astype(bf16)
    src_f = np.asarray(src).ravel().astype(np.int64)
    dst_f = np.asarray(dst).ravel().astype(np.int64)
    assert src_f.shape == (E,) and dst_f.shape == (E,)

    order = np.argsort(dst_f, kind="stable")
    per_core = []
    for jc in range(NCORES):
        sl = order[jc * ECORE : (jc + 1) * ECORE]
        per_core.append(_chunk_core(src_f[sl], dst_f[sl], sl))

    nchunk = _cdiv(max(pc[0].shape[0] for pc in per_core), CPC) * CPC
    g3 = _cdiv(nchunk, 3)

    if nchunk not in _PROGRAM_CACHE:
        _PROGRAM_CACHE[nchunk] = _build_program(nchunk)
    nc = _PROGRAM_CACHE[nchunk]

    in_maps = []
    eid_list = []
    for jc in range(NCORES):
        srcs, cells, jloc, eids = per_core[jc]
        m = srcs.shape[0]
        pad3 = g3 * 3  # chunk slots incl. packing remainder
        if m < pad3:
            pad = pad3 - m
            srcs = np.vstack([srcs, np.zeros((pad, P), np.int16)])
            cells = np.vstack([cells, np.zeros((pad, W), np.int32)])
            jloc = np.vstack([jloc, np.full((pad, P), -1, np.int8)])
            eids = np.vstack([eids, np.full((pad, P), -1, np.int64)])
        # slab: [96, g3, 256]; chunk c -> partitions 32*(c%3)+w, column c//3
        rows = ifeat_bf[cells.ravel()].reshape(pad3, W, D)
        slabT = np.ascontiguousarray(
            rows.reshape(g3, 3, W, D).transpose(1, 2, 0, 3).reshape(96, g3, D)
        )
        # onehot^T: [96, g3, 128]; oht[32*(c%3)+w, c//3, e] = (jloc[c,e] == w)
        ohb = (
            jloc[:, :, None] == np.arange(W, dtype=np.int8)[None, None, :]
        ).transpose(0, 2, 1)                       # [pad3, W, 128]
        ohT = np.ascontiguousarray(
            ohb.reshape(g3, 3, W, P).transpose(1, 2, 0, 3).reshape(96, g3, P)
        ).astype(ml_dtypes.float8_e4m3)
        sidx_w = _wrap_idx(srcs[:nchunk].ravel())
        in_maps.append(
            {"ufeat": ufeat_bf, "slab": slabT, "oht": ohT, "sidx": sidx_w}
        )
        eid_list.append(eids[:nchunk])

    res = run_bass_kernel_spmd(nc, in_maps, core_ids=list(range(NCORES)))

    out = np.empty((E, 1), np.float32)
    for jc in range(NCORES):
        yj = np.asarray(res.results[jc]["y"])  # [128, nchunk]
        eids = eid_list[jc]                    # [nchunk, 128]
        m = eids >= 0
        out[eids[m], 0] = yj.T[m]
    return out


# revision 24
# speedup vs baseline: 1.1961x; 1.0150x over previous
"""Trainium2 Bass kernel for BiDecoder edge dot products.

out[e] = dot(ufeat[src[e]], ifeat[dst[e]])   for E=300000 edges, D=256.

Strategy (8 NeuronCores, SPMD, v4 = dst-slab + PE one-hot expansion, flat gather):
  - Edges are globally sorted by dst; each core takes a contiguous 37500-edge
    slice, grouped into chunks of <=128 edges touching <=32 distinct cells.
    The host packs each chunk's distinct cells' ifeat rows (bf16) into a
    per-chunk 32-row window of a slab (3 chunks share a 128-partition block at
    base partitions 0/32/64 - the PE's allowed stationary bases), so the whole
    ifeat side needs NO per-edge gather, just a contiguous ~5MB slab load.
  - hu rows are gathered per-edge from HBM in bf16 (512B rows) with flat
    (transpose=False) SWDGE dma_gather on 4 queues. (Transpose-mode gathers
    corrupt data when calls run concurrently across queues - measured - so
    the kernel keeps hu in [edge, d] layout and never transposes it.)
  - PE: per chunk, HVexp[e, :] = onehot^T @ window  (K=32 window rows,
    M=128 edges, N=256 features) - one matmul expands the chunk's hv rows
    into PSUM, selecting each edge's own dst row.
  - DVE: per chunk, affine_mul_reduce(hu * HVexp) accumulates the per-edge
    dot into osb[:, c].
  - All cross-engine handoffs keep >=1 call/chunk of slack beyond the exact
    dependency boundary: semaphore updates can lead actual engine progress
    (measured on HW), so exact-boundary pipelining corrupts.
  - Host reorders y back to original edge order.
"""

import sys

for _p in ("/opt/trn_rl_repo",):
    if _p not in sys.path:
        sys.path.append(_p)

import numpy as np

P = 128
D = 256
E = 300000
NCORES = 8
ECORE = E // NCORES
N_GENE = 20000
N_CELL = 50000
W = 32                   # distinct cells per chunk window
CPC = 8                  # chunks per gather call (1024 rows/call)
NSLOT = 8                # gather slots (multiple of 4: slot sem <-> SWDGE queue binding)
NPS = 6                  # PSUM expansion tiles in rotation
NPROD = 4                # DVE product scratch tiles
NSTRIPE = 8              # slab/onehot load stripes

_PROGRAM_CACHE: dict = {}


def _cdiv(a, b):
    return -(-a // b)


def _wrap_idx(idx_i16: np.ndarray) -> np.ndarray:
    """[n] int16 -> [128, n/16] dma_gather idx layout (16-wrapped, 8x tiled)."""
    n = idx_i16.shape[0]
    assert n % 128 == 0
    cols = n // 16
    w = idx_i16.reshape(cols, 16).T
    return np.ascontiguousarray(np.tile(w, (8, 1)))


def _build_program(nchunk):
    import concourse.bacc as bacc
    import concourse.mybir as mybir
    from concourse.library_config import mlp

    assert nchunk % CPC == 0
    ncall = nchunk // CPC
    g3 = _cdiv(nchunk, 3)
    gs = _cdiv(g3 - 2, NSTRIPE - 1)
    stripes = [(0, 2)] + [
        (2 + i * gs, min(g3, 2 + (i + 1) * gs)) for i in range(NSTRIPE - 1)
    ]
    stripe_of_col = {}
    for i, (c0, c1) in enumerate(stripes):
        for c in range(c0, c1):
            stripe_of_col[c] = i
    bf16 = mybir.dt.bfloat16
    f32 = mybir.dt.float32

    nc = bacc.Bacc("TRN2", debug=False, num_swdge_queues=4,
                   dynamic_dma_scratch_size=32768)
    ufeat = nc.dram_tensor("ufeat", [N_GENE, D], bf16, kind="ExternalInput")
    slab = nc.dram_tensor("slab", [96, g3, D], bf16, kind="ExternalInput")
    oht = nc.dram_tensor("oht", [96, g3, P], mybir.dt.float8e4,
                         kind="ExternalInput")
    sidx = nc.dram_tensor("sidx", [P, nchunk * P // 16], mybir.dt.int16,
                          kind="ExternalInput")
    y = nc.dram_tensor("y", [P, nchunk], f32, kind="ExternalOutput")

    with (
        nc.sbuf_tensor("hu", [P, NSLOT, CPC, D], bf16) as hu,
        nc.sbuf_tensor("slab_sb", [P, g3, D], bf16) as slab_sb,
        nc.sbuf_tensor("oht_sb", [P, g3, P], mybir.dt.float8e4) as oht_sb,
        nc.sbuf_tensor("sidx_sb", [P, nchunk * P // 16], mybir.dt.int16) as sidx_sb,
        nc.sbuf_tensor("prod", [P, NPROD, D], bf16) as prod,
        nc.sbuf_tensor("osb", [P, nchunk], f32) as osb,
        nc.psum_tensor("hv0", [P, D], f32) as hv0,
        nc.psum_tensor("hv1", [P, D], f32) as hv1,
        nc.psum_tensor("hv2", [P, D], f32) as hv2,
        nc.psum_tensor("hv3", [P, D], f32) as hv3,
        nc.psum_tensor("hv4", [P, D], f32) as hv4,
        nc.psum_tensor("hv5", [P, D], f32) as hv5,
        nc.psum_tensor("hvdum", [P, D], f32) as hvdum,
        nc.semaphore("io_s") as io_s,
        nc.semaphore("io") as io,
        nc.semaphore("pec") as pec,
        nc.semaphore("dvec") as dvec,
        nc.semaphore("io2") as io2,
        nc.Block(no_gpsimd_drain=True) as block,
        __import__("contextlib").ExitStack() as _stk,
    ):
        gu = [_stk.enter_context(nc.semaphore(f"gu{i}")) for i in range(NSLOT)]
        hvps = [hv0, hv1, hv2, hv3, hv4, hv5]

        @block.gpsimd
        def _(gp):
            gp.load_library(mlp)
            idxcols = nchunk * P // 16
            q4 = _cdiv(idxcols, 4)
            lastq = -1
            for k in range(ncall):
                kq = min(3, ((k + 1) * CPC * P // 16 - 1) // q4)
                if kq > lastq:
                    gp.wait_ge(io_s, 16 * (kq + 1))
                    lastq = kq
                s = k % NSLOT
                if k >= NSLOT - 1:
                    # slot freed when DVE consumed call k-NSLOT; +1 call slack
                    gp.wait_ge(dvec, CPC * (k - NSLOT + 2))
                cols = slice(k * CPC * P // 16, (k + 1) * CPC * P // 16)
                gp.dma_gather(
                    hu[:, s], ufeat[:, :], sidx_sb[:, cols],
                    CPC * P, CPC * P, D,
                    transpose=False, queue_num=k % 4, single_packet=False,
                ).then_inc(gu[s], 16)
            for s in range(NSLOT):
                cnt = (ncall - s + NSLOT - 1) // NSLOT
                if cnt:
                    gp.wait_ge(gu[s], 16 * cnt)

        @block.tensor
        def _(pe):
            laststripe = -1
            for c in range(nchunk):
                stripe = stripe_of_col[c // 3]
                if stripe > laststripe:
                    pe.wait_ge(io, 32 * (stripe + 1))
                    laststripe = stripe
                if c >= NPS:
                    # psum tile c%NPS freed when DVE did chunk c-NPS; +1 slack
                    pe.wait_ge(dvec, max(1, c - NPS + 2))
                p0 = 32 * (c % 3)
                g = c // 3
                pe.matmul(
                    out=hvps[c % NPS][:, :],
                    lhsT=oht_sb[p0 : p0 + W, g, :],
                    rhs=slab_sb[p0 : p0 + W, g, :],
                    start=True, stop=True,
                ).then_inc(pec, 1)
            for _ in range(4):
                pe.matmul(
                    out=hvdum[:, :],
                    lhsT=oht_sb[0:W, 0, :],
                    rhs=slab_sb[0:W, 0, :],
                    start=True, stop=True,
                ).then_inc(pec, 1)

        @block.vector
        def _(v):
            for c in range(nchunk):
                k, r = divmod(c, CPC)
                s = k % NSLOT
                if r == 0:
                    v.wait_ge(gu[s], 16 * (k // NSLOT + 1))
                # chunk c's expansion executed: PE issued through c+2 (slack)
                v.wait_ge(pec, c + 3)
                if c >= NPROD:
                    v.wait_ge(dvec, c - NPROD + 1)  # prod scratch WAR self-gate
                v.affine_mul_reduce(
                    out=prod[:, c % NPROD, :],
                    accum_out=osb[:, c : c + 1],
                    in0=hu[:, s, r, :],
                    in1=hvps[c % NPS][:, :],
                    scale=1.0,
                    bias=0.0,
                ).then_inc(dvec, 1)

        @block.scalar
        def _(sc):
            idxcols = nchunk * P // 16
            q4 = _cdiv(idxcols, 4)
            for i in range(4):
                if i > 0:
                    sc.wait_ge(io_s, 16 * i)
                cs = slice(i * q4, min(idxcols, (i + 1) * q4))
                sc.dma_start(sidx_sb[:, cs], sidx[:, cs]).then_inc(io_s, 16)

        @block.sync
        def _(sy):
            # striped slab/onehot loads: tiny first stripe so PE starts fast;
            # later stripes wait for DVE progress so their traffic never
            # competes with the early gather calls
            for i, (c0, c1) in enumerate(stripes):
                if i > 0:
                    sy.wait_ge(io, 32 * i)
                    sy.wait_ge(dvec, max(0, 8 * (i - 1)))
                cs = slice(c0, c1)
                sy.dma_start(slab_sb[0:96, cs], slab[:, cs]).then_inc(io, 16)
                sy.dma_start(oht_sb[0:96, cs], oht[:, cs]).then_inc(io, 16)
            sy.wait_ge(dvec, nchunk)
            sy.dma_start(y[:, :], osb[:, :]).then_inc(io2, 16)
            sy.wait_ge(io2, 16)

    nc.compile()
    return nc


def _chunk_core(s_j, d_j, ids_j):
    """Greedy chunks: <=128 edges, <=W distinct (sorted) dst cells each.

    Returns (srcs [n,128] int16 pad0, cells [n,W] int32 pad0,
             j [n,128] int8 pad-1, eids [n,128] int64 pad-1).
    """
    n = len(d_j)
    newcell = np.empty(n, bool)
    newcell[0] = True
    np.not_equal(d_j[1:], d_j[:-1], out=newcell[1:])
    cellrank = np.cumsum(newcell) - 1
    chunks = []
    i = 0
    while i < n:
        base_rank = cellrank[i]
        hi = min(n, i + 128)
        if cellrank[hi - 1] - base_rank >= W:
            hi = int(np.searchsorted(cellrank[i:hi], base_rank + W)) + i
        chunks.append((i, hi))
        i = hi
    m = len(chunks)
    srcs = np.zeros((m, P), np.int16)
    cells = np.zeros((m, W), np.int32)
    jloc = np.full((m, P), -1, np.int8)
    eids = np.full((m, P), -1, np.int64)
    for ci, (lo, hi) in enumerate(chunks):
        k = hi - lo
        srcs[ci, :k] = s_j[lo:hi]
        jloc[ci, :k] = (cellrank[lo:hi] - cellrank[lo]).astype(np.int8)
        eids[ci, :k] = ids_j[lo:hi]
        cc = np.unique(d_j[lo:hi])
        cells[ci, : len(cc)] = cc
    return srcs, cells, jloc, eids


def kernel(ufeat, ifeat, src, dst):
    import ml_dtypes
    from concourse.bass_utils import run_bass_kernel_spmd

    bf16 = ml_dtypes.bfloat16
    ufeat_bf = np.ascontiguousarray(np.asarray(ufeat, dtype=np.float32)).astype(bf16)
    ifeat_bf = np.ascontiguousarray(np.asarray(ifeat, dtype=np.float32)).astype(bf16)
    src_f = np.asarray(src).ravel().astype(np.int64)
    dst_f = np.asarray(dst).ravel().astype(np.int64)
    assert src_f.shape == (E,) and dst_f.shape == (E,)

    order = np.argsort(dst_f, kind="stable")
    per_core = []
    for jc in range(NCORES):
        sl = order[jc * ECORE : (jc + 1) * ECORE]
        per_core.append(_chunk_core(src_f[sl], dst_f[sl], sl))

    nchunk = _cdiv(max(pc[0].shape[0] for pc in per_core), CPC) * CPC
    g3 = _cdiv(nchunk, 3)

    if nchunk not in _PROGRAM_CACHE:
        _PROGRAM_CACHE[nchunk] = _build_program(nchunk)
    nc = _PROGRAM_CACHE[nchunk]

    in_maps = []
    eid_list = []
    for jc in range(NCORES):
        srcs, cells, jloc, eids = per_core[jc]
        m = srcs.shape[0]
        pad3 = g3 * 3  # chunk slots incl. packing remainder
        if m < pad3:
            pad = pad3 - m
            srcs = np.vstack([srcs, np.zeros((pad, P), np.int16)])
            cells = np.vstack([cells, np.zeros((pad, W), np.int32)])
            jloc = np.vstack([jloc, np.full((pad, P), -1, np.int8)])
            eids = np.vstack([eids, np.full((pad, P), -1, np.int64)])
        # slab: [96, g3, 256]; chunk c -> partitions 32*(c%3)+w, column c//3
        rows = ifeat_bf[cells.ravel()].reshape(pad3, W, D)
        slabT = np.ascontiguousarray(
            rows.reshape(g3, 3, W, D).transpose(1, 2, 0, 3).reshape(96, g3, D)
        )
        # onehot^T: [96, g3, 128]; oht[32*(c%3)+w, c//3, e] = (jloc[c,e] == w)
        ohb = (
            jloc[:, :, None] == np.arange(W, dtype=np.int8)[None, None, :]
        ).transpose(0, 2, 1)                       # [pad3, W, 128]
        ohT = np.ascontiguousarray(
            ohb.reshape(g3, 3, W, P).transpose(1, 2, 0, 3).reshape(96, g3, P)
        ).astype(ml_dtypes.float8_e4m3)
        sidx_w = _wrap_idx(srcs[:nchunk].ravel())
        in_maps.append(
            {"ufeat": ufeat_bf, "slab": slabT, "oht": ohT, "sidx": sidx_w}
        )
        eid_list.append(eids[:nchunk])

    res = run_bass_kernel_spmd(nc, in_maps, core_ids=list(range(NCORES)))

    out = np.empty((E, 1), np.float32)
    for jc in range(NCORES):
        yj = np.asarray(res.results[jc]["y"])  # [128, nchunk]
        eids = eid_list[jc]                    # [nchunk, 128]
        m = eids >= 0
        out[eids[m], 0] = yj.T[m]
    return out
